# revision 1
# baseline (speedup 1.0000x reference)
"""Trainium2 Bass kernel: GNN edge decoder (nn_Decoder).

Computation (per edge e):
    emb  = concat(X[src[e]], X[dst[e]])          # [256]
    h    = relu(emb @ W1.T + b1)                 # [128]
    logit= h @ W2.T + b2                         # scalar
Outputs: (logits[E], labels[E]=ones)

Device strategy (8 cores, data-parallel over edges):

Phase 1 (Tile, replicated): precompute node table in fp16
    P[n] = [ X[n] @ W1a.T + b1  ||  X[n] @ W1b.T ]     ([N,256], row=512B)
  via per-tile PE transpose of X + one matmul with [W1a.T||W1b.T], bias folded
  in with a K=1 accumulate matmul.

Phase 2 (raw bass): per-edge gather + MLP tail.
  The only fast gather on TRN2 is the GPSIMD CounterMachine `dma_gather`
  (int16 indices, <=1024 per call), so the HOST bins each core's edges into
  16 groups by (src_range, dst_range) over 4 node ranges of 25024 rows;
  within a group both gathers use bin-local int16 indices. Groups are padded
  to a fixed quota (7 calls x 1024). Per 1024-edge unit:
    dma_gather A <- P[src_bin rows, 0:128], dma_gather B <- P[dst_bin, 128:256]
    DVE: s = A + B;  prod = max(s,0) * w2;  logits = sum(prod) + b2
    ACT: DMA logits out.
  Host un-permutes logits and drops pad slots.
"""

import numpy as np

D = 128
N_NODES = 100000
E_TOTAL = 800000
N_CORES = 8

FULL_CFG = dict(
    n_pad=100096,      # 782*128 node rows (padded)
    chunk_t=34,        # node tiles per precompute chunk
    n_chunks=23,       # 23*34*128 = 100096
    bin_rows=25024,    # node rows per bin (4 bins)
    qg=1024,           # indices per dma_gather call (hw limit)
    calls_per_group=7, # quota = 7*1024 = 7168 slots per group
    depth=14,          # gather units in flight (must divide 112)
    lbuf=8,            # logits tiles in flight
)
N_GROUPS = 16


def _units(cfg):
    return N_GROUPS * cfg["calls_per_group"]


def _slots(cfg):
    return _units(cfg) * cfg["qg"]


def build_bass(cfg=None, n_reps=1, n_reps_p1=None, n_reps_p2=None, tail=True):
    from contextlib import ExitStack

    import concourse.bacc as bacc
    import concourse.tile as tile
    from concourse import bass, library_config, mybir
    from concourse.masks import make_identity

    cfg = cfg or FULL_CFG
    n_pad = cfg["n_pad"]
    chunk_t = cfg["chunk_t"]
    n_chunks = cfg["n_chunks"]
    BINR = cfg["bin_rows"]
    QG = cfg["qg"]
    CPG = cfg["calls_per_group"]
    DEPTH = cfg["depth"]
    LBUF = cfg["lbuf"]
    NU = _units(cfg)
    assert n_chunks * chunk_t * 128 == n_pad
    assert 4 * BINR == n_pad
    assert QG % 128 == 0 and QG <= 1024
    assert NU % DEPTH == 0 and NU % LBUF == 0
    JPU = QG // 128           # logits free cols per unit
    SPU = QG // 16            # idx cols per call

    fp16 = mybir.dt.float16
    f32 = mybir.dt.float32
    i16 = mybir.dt.int16
    ALU = mybir.AluOpType

    nc = bacc.Bacc(
        "TRN2", target_bir_lowering=False, debug=False, num_devices=N_CORES,
        num_swdge_queues=2, dynamic_dma_scratch_size=32768,
    )

    x_d = nc.dram_tensor("x", [n_pad, D], f32, kind="ExternalInput").ap()
    w1_d = nc.dram_tensor("w1", [D, 2 * D], f32, kind="ExternalInput").ap()
    b1_d = nc.dram_tensor("b1", [D], f32, kind="ExternalInput").ap()
    w2_d = nc.dram_tensor("w2", [1, D], f32, kind="ExternalInput").ap()
    b2_d = nc.dram_tensor("b2", [1], f32, kind="ExternalInput").ap()
    gidx_d = nc.dram_tensor("gidx", [128, NU * 2 * SPU], i16, kind="ExternalInput").ap()
    out_d = nc.dram_tensor("logits", [_slots(cfg)], f32, kind="ExternalOutput").ap()
    p_d = nc.dram_tensor("ptab", [n_pad, 2 * D], fp16).ap()

    # persistent SBUF for phase-2 (written during phase 1)
    w2rep_t = nc.alloc_sbuf_tensor("w2rep", [128, D], fp16)
    b2bc_t = nc.alloc_sbuf_tensor("b2bc", [128, 1], f32)
    idx_all_t = nc.alloc_sbuf_tensor("idx_all", [128, NU * 2 * SPU], i16)
    dstA_t = nc.alloc_sbuf_tensor("dstA", [128, DEPTH * JPU, 128], fp16)
    dstB_t = nc.alloc_sbuf_tensor("dstB", [128, DEPTH * JPU, 128], fp16)
    red1_t = nc.alloc_sbuf_tensor("red1", [128, JPU * 16], fp16)
    lg_t = nc.alloc_sbuf_tensor("lg", [128, LBUF * JPU], f32)
    w2rep = w2rep_t.ap()
    b2bc = b2bc_t.ap()
    idx_all = idx_all_t.ap()
    dstA = dstA_t.ap()
    dstB = dstB_t.ap()
    red1 = red1_t.ap()
    lg = lg_t.ap()

    if n_reps_p1 is None:
        n_reps_p1 = n_reps
    if n_reps_p2 is None:
        n_reps_p2 = n_reps

    with ExitStack() as top:
        idx_sem = top.enter_context(nc.semaphore("idx_sem"))
        dv = top.enter_context(nc.semaphore("dv"))
        gsems = [
            top.enter_context(nc.semaphore(f"gs{k}")) for k in range(DEPTH)
        ]
        olsems = [
            top.enter_context(nc.semaphore(f"ol{k}")) for k in range(LBUF)
        ]

        for rep in range(n_reps_p1):
            # ---------------- phase 1 (Tile): precompute P -----------------
            with ExitStack() as ctx:
                tc = ctx.enter_context(tile.TileContext(nc))
                const = ctx.enter_context(tc.tile_pool(name="const", bufs=1))

                ident = const.tile([128, 128], fp16)
                make_identity(nc, ident[:])

                w1f = const.tile([128, 2 * D], f32)
                nc.sync.dma_start(out=w1f[:], in_=w1_d)
                w1h = const.tile([128, 2 * D], fp16)
                nc.vector.tensor_copy(out=w1h[:], in_=w1f[:])
                wcatT = const.tile([128, 2 * D], fp16)

                b1f = const.tile([1, D], f32)
                nc.sync.dma_start(out=b1f[:], in_=b1_d.unsqueeze(0))
                b1cat = const.tile([1, 2 * D], fp16)
                nc.vector.memset(b1cat[:], 0.0)
                nc.vector.tensor_copy(out=b1cat[:, 0:D], in_=b1f[:])

                w2f = const.tile([1, D], f32)
                nc.sync.dma_start(out=w2f[:], in_=w2_d)
                w2h = const.tile([1, D], fp16)
                nc.vector.tensor_copy(out=w2h[:], in_=w2f[:])
                b2f = const.tile([1, 1], f32)
                nc.sync.dma_start(out=b2f[:], in_=b2_d.unsqueeze(0))

                onesh = const.tile([1, D], fp16)
                nc.vector.memset(onesh[:], 1.0)
                ones32 = const.tile([1, 128], f32)
                nc.vector.memset(ones32[:], 1.0)

                with tc.tile_pool(name="ps_setup", bufs=1, space="PSUM") as pss:
                    for half in range(2):
                        tp = pss.tile([128, 128], fp16, tag="t")
                        nc.tensor.transpose(
                            tp[:], w1h[:, half * 128 : (half + 1) * 128], ident[:]
                        )
                        nc.scalar.copy(wcatT[:, half * 128 : (half + 1) * 128], tp[:])

                    wps = pss.tile([128, D], f32, tag="b")
                    nc.tensor.matmul(
                        wps[:], lhsT=onesh[:], rhs=w2h[:], start=True, stop=True
                    )
                    nc.vector.tensor_copy(out=w2rep, in_=wps[:])

                    b2ps = pss.tile([128, 1], f32, tag="s")
                    nc.tensor.matmul(
                        b2ps[:], lhsT=ones32[:], rhs=b2f[:], start=True, stop=True
                    )
                    nc.vector.tensor_copy(out=b2bc, in_=b2ps[:])

                xw_pool = ctx.enter_context(tc.tile_pool(name="xw", bufs=2))
                pw_pool = ctx.enter_context(tc.tile_pool(name="pw", bufs=2))
                xt_pool = ctx.enter_context(tc.tile_pool(name="xt", bufs=3))
                psA = ctx.enter_context(tc.tile_pool(name="psA", bufs=3, space="PSUM"))
                psB = ctx.enter_context(tc.tile_pool(name="psB", bufs=3, space="PSUM"))

                rpc = chunk_t * 128
                for c in range(n_chunks):
                    xw = xw_pool.tile([128, chunk_t * 128], fp16, tag="xw")
                    src_rows = x_d[c * rpc : (c + 1) * rpc, :]
                    nc.gpsimd.dma_start(
                        out=xw[:].rearrange("p (t f) -> p t f", f=128),
                        in_=src_rows.rearrange("(t p) f -> p t f", p=128),
                    )
                    pw = pw_pool.tile([128, chunk_t * 256], fp16, tag="pw")
                    for t in range(chunk_t):
                        xt_ps = psA.tile([128, 128], fp16, tag="xt_ps")
                        nc.tensor.transpose(
                            xt_ps[:], xw[:, t * 128 : (t + 1) * 128], ident[:]
                        )
                        xt_sb = xt_pool.tile([128, 128], fp16, tag="xt_sb")
                        nc.scalar.copy(xt_sb[:], xt_ps[:])
                        pp = psB.tile([128, 256], f32, tag="pp")
                        nc.tensor.matmul(
                            pp[:], lhsT=xt_sb[:], rhs=wcatT[:], start=True, stop=False
                        )
                        nc.tensor.matmul(
                            pp[:], lhsT=onesh[:], rhs=b1cat[:], start=False, stop=True
                        )
                        if t % 2 == 0:
                            nc.vector.tensor_copy(pw[:, t * 256 : (t + 1) * 256], pp[:])
                        else:
                            nc.scalar.copy(pw[:, t * 256 : (t + 1) * 256], pp[:])
                    dst_rows = p_d[c * rpc : (c + 1) * rpc, :]
                    nc.sync.dma_start(
                        out=dst_rows.rearrange("(t p) f -> p t f", p=128),
                        in_=pw[:].rearrange("p (t f) -> p t f", f=256),
                    )

        for rep in range(n_reps_p2):
            # ---------------- phase 2 (raw): gather + tail -----------------
            w2b3 = w2rep.unsqueeze(1).to_broadcast([128, JPU, 128])
            base_i = rep * 16           # idx_sem: one bulk load per rep
            base_d = rep * NU
            ng_slot = NU // DEPTH
            nl_slot = NU // LBUF
            base_g = rep * 32 * ng_slot   # per gsems slot, 32 per use
            base_o = rep * 16 * nl_slot   # per olsems slot, 16 per use

            with nc.Block() as block:

                @block.gpsimd
                def _(gp, rep=rep, base_g=base_g, base_d=base_d, base_i=base_i):
                    gp.load_library(library_config.mlp)
                    gp.wait_ge(idx_sem, base_i + 16)
                    for u in range(NU):
                        gr = u // CPG
                        sb, db = gr // 4, gr % 4
                        k = u % DEPTH
                        if u >= DEPTH:
                            gp.wait_ge(dv, base_d + u - DEPTH + 1)
                        acol = (2 * u) * SPU
                        bcol = (2 * u + 1) * SPU
                        gp.dma_gather(
                            dstA[:, k * JPU : (k + 1) * JPU, :],
                            p_d[sb * BINR : (sb + 1) * BINR, 0:128],
                            idx_all[:, acol : acol + SPU],
                            QG, QG, 128, elem_step=256, single_packet=False,
                            queue_num=0,
                        ).then_inc(gsems[k], 16)
                        gp.dma_gather(
                            dstB[:, k * JPU : (k + 1) * JPU, :],
                            p_d[db * BINR : (db + 1) * BINR, 128:256],
                            idx_all[:, bcol : bcol + SPU],
                            QG, QG, 128, elem_step=256, single_packet=False,
                            queue_num=1,
                        ).then_inc(gsems[k], 16)
                    gp.wait_ge(dv, base_d + NU)

                @block.vector
                def _(vec, rep=rep, base_g=base_g, base_d=base_d, base_o=base_o):
                    if not tail:
                        for u in range(NU):
                            k = u % DEPTH
                            vec.wait_ge(gsems[k], base_g + 32 * (u // DEPTH + 1))
                            vec.drain().then_inc(dv, 1)
                        return
                    for u in range(NU):
                        k = u % DEPTH
                        lk = u % LBUF
                        vec.wait_ge(gsems[k], base_g + 32 * (u // DEPTH + 1))
                        sA = dstA[:, k * JPU : (k + 1) * JPU, :]
                        sB = dstB[:, k * JPU : (k + 1) * JPU, :]
                        vec.tensor_add(out=sA, in0=sA, in1=sB)
                        vec.drain()
                        vec.scalar_tensor_tensor(
                            out=sB, in0=sA, scalar=0.0, in1=w2b3,
                            op0=ALU.max, op1=ALU.mult,
                        )
                        vec.drain()
                        if u >= LBUF:
                            vec.wait_ge(olsems[lk], base_o + 16 * (u // LBUF))
                        with nc.allow_low_precision("fp16 partial reduce, 8 terms"):
                            vec.tensor_reduce(
                                out=red1[:].rearrange("p (j s) -> p j s", s=16),
                                in_=sB.rearrange("p j (s w) -> p j s w", w=8),
                                axis=mybir.AxisListType.X,
                                op=ALU.add,
                            )
                        vec.drain()
                        lslice = lg[:, lk * JPU : (lk + 1) * JPU]
                        vec.tensor_reduce(
                            out=lslice,
                            in_=red1[:].rearrange("p (j s) -> p j s", s=16),
                            axis=mybir.AxisListType.X,
                            op=ALU.add,
                        )
                        vec.drain()
                        vec.tensor_scalar_add(
                            out=lslice, in0=lslice, scalar1=b2bc[:, 0:1]
                        ).then_inc(dv, 1)
                    for lk in range(LBUF):
                        vec.wait_ge(olsems[lk], base_o + 16 * (NU // LBUF))

                @block.scalar
                def _(act, rep=rep, base_d=base_d, base_i=base_i, base_o=base_o):
                    act.dma_start(out=idx_all, in_=gidx_d).then_inc(idx_sem, 16)
                    if not tail:
                        act.wait_ge(dv, base_d + NU)
                        return
                    for u in range(NU):
                        lk = u % LBUF
                        act.wait_ge(dv, base_d + u + 1)
                        act.dma_start(
                            out=out_d[u * QG : (u + 1) * QG].rearrange(
                                "(p j) -> p j", p=128
                            ),
                            in_=lg[:, lk * JPU : (lk + 1) * JPU],
                        ).then_inc(olsems[lk], 16)
                    for lk in range(LBUF):
                        act.wait_ge(olsems[lk], base_o + 16 * (NU // LBUF))

    nc.compile()
    return nc


def make_in_maps(inputs, cfg=None, n_cores=N_CORES):
    """Shard, bin, and pad host inputs into per-core input maps.

    Returns (in_maps, pos_list) where pos_list[c] maps each original edge of
    core c to its device slot in the logits output.
    """
    cfg = cfg or FULL_CFG
    n_pad = cfg["n_pad"]
    BINR = cfg["bin_rows"]
    QG = cfg["qg"]
    CPG = cfg["calls_per_group"]
    NU = _units(cfg)
    SPU = QG // 16
    quota = CPG * QG

    x = np.asarray(inputs["block_outputs"], dtype=np.float32)
    n_nodes = x.shape[0]
    x_pad = np.zeros((n_pad, D), dtype=np.float32)
    x_pad[:n_nodes] = x

    src = np.asarray(inputs["src"]).astype(np.int64)
    dst = np.asarray(inputs["dst"]).astype(np.int64)
    e_total = src.shape[0]
    e_core = e_total // n_cores

    w1 = np.ascontiguousarray(np.asarray(inputs["W1"], dtype=np.float32))
    b1 = np.ascontiguousarray(np.asarray(inputs["b1"], dtype=np.float32))
    w2 = np.ascontiguousarray(np.asarray(inputs["W2"], dtype=np.float32))
    b2 = np.ascontiguousarray(np.asarray(inputs["b2"], dtype=np.float32))

    in_maps, pos_list = [], []
    for c in range(n_cores):
        s_c = src[c * e_core : (c + 1) * e_core]
        d_c = dst[c * e_core : (c + 1) * e_core]
        grp = (s_c // BINR) * 4 + (d_c // BINR)
        order = np.argsort(grp, kind="stable")
        counts = np.bincount(grp, minlength=16)
        assert counts.max() <= quota, f"group quota exceeded: {counts.max()}"

        # per-group padded local indices + device position of each edge
        gidx = np.zeros((128, NU * 2 * SPU), dtype=np.int16)
        pos = np.empty(e_core, dtype=np.int64)
        off = 0
        for gr in range(16):
            cnt = counts[gr]
            eids = order[off : off + cnt]
            off += cnt
            sl = np.zeros(quota, dtype=np.int16)
            dl = np.zeros(quota, dtype=np.int16)
            sl[:cnt] = (s_c[eids] - (gr // 4) * BINR).astype(np.int16)
            dl[:cnt] = (d_c[eids] - (gr % 4) * BINR).astype(np.int16)
            i = np.arange(cnt)
            u_loc = i // QG
            k = i % QG
            pos[eids] = (gr * CPG + u_loc) * QG + (k % 128) * (QG // 128) + k // 128
            # write wrapped idx tiles for each call of this group
            for cc in range(CPG):
                u = gr * CPG + cc
                a = sl[cc * QG : (cc + 1) * QG].reshape(QG // 16, 16).T
                b = dl[cc * QG : (cc + 1) * QG].reshape(QG // 16, 16).T
                gidx[:, (2 * u) * SPU : (2 * u + 1) * SPU] = np.tile(a, (8, 1))
                gidx[:, (2 * u + 1) * SPU : (2 * u + 2) * SPU] = np.tile(b, (8, 1))

        in_maps.append(
            {
                "x": x_pad, "w1": w1, "b1": b1, "w2": w2, "b2": b2,
                "gidx": np.ascontiguousarray(gidx),
            }
        )
        pos_list.append(pos)
    return in_maps, pos_list


_COMPILED = None


def kernel(**inputs):
    """Full-input entry point: shards across 8 NeuronCores, returns full output."""
    global _COMPILED
    from concourse.bass_utils import run_bass_kernel_spmd

    if _COMPILED is None:
        _COMPILED = build_bass(FULL_CFG)
    nc = _COMPILED

    in_maps, pos_list = make_in_maps(inputs, FULL_CFG)
    res = run_bass_kernel_spmd(nc, in_maps, core_ids=list(range(N_CORES))).results
    logits = np.concatenate(
        [res[c]["logits"][pos_list[c]] for c in range(N_CORES)]
    ).astype(np.float32)
    labels = np.ones_like(logits)
    return logits, labels

